# revision 29
# baseline (speedup 1.0000x reference)
"""Trainium2 Bass kernel for nn_EquivariantBinaryClassificationSAGPoolScalar.

Algebraic reduction of the reference (per graph g):
  z=x@out_w, xs1=x@sc_w1+sc_b1, y2=x@sc_w2   (per-node scalars)
  W1=ea@dp_w1+dp_b1, W2=ea@dp_w2+dp_b2       (per-edge scalars)
  score1 = segment-mean over dst of xs1[src]*W1
  kept1 = top-512/graph (threshold = 512th largest), t1 = tanh(score1)
  m = kept1*(y2*t1 + sc_b2)
  score2 = segment-mean over dst of m[src]*W2 with count of (m[src] != 0)
  kept2 = top-256 among kept1 by score2, t2 = tanh(score2)
  out_g = sigmoid(sum_i z_i*(1 + kept1*t1*(1 + kept2*t2)) + out_b)

Sharding: 8 graphs per core (contiguous slices). Device: PE projections +
PE bilinear segment-sum (32x32 one-hot factorization, bf16 one-hots/messages
with f32 PSUM accumulate), one gpsimd ap_gather (16384 idxs) per layer for
xs[src], and exact per-graph top-k thresholds via 40-step count-target
bisection on DVE (is_ge + reduce) with PE matmuls for the tiny cross-partition
count sums - replaces 16 serial gpsimd kth_largest calls (~117us each).

Edge-slot enumeration: slot (p, s) holds edge e = 1024*(s//8) + 8*p + (s%8);
graph g owns slots s in [128g, 128g+128).
"""
import sys
import numpy as np

if "/opt/trn_rl_repo" not in sys.path:
    sys.path.insert(0, "/opt/trn_rl_repo")

import concourse.bass as bass
import concourse.bacc as bacc
import concourse.mybir as mybir
import concourse.tile as tile
from concourse.masks import make_identity
from concourse.bass_utils import run_bass_kernel_spmd

F32 = mybir.dt.float32
BF16 = mybir.dt.bfloat16
I32 = mybir.dt.int32
I16 = mybir.dt.int16
I8 = mybir.dt.int8
AL = mybir.AluOpType
ACTF = mybir.ActivationFunctionType

G = 8
NPG = 1024
NN = G * NPG
EPG = 16 * NPG
E = G * EPG
C = 256
EC = 48
K1 = NPG // 2
K2 = NPG // 4
NCOL = NN // 128
SLOTS = E // 128




def _ap(t, off_elems, free_dims):
    a = t[:]
    return bass.AP(a.tensor, a.offset + off_elems, [list(a.ap[0])] + free_dims)


def build_program(debug=False, reps=1, stage=99):
    nc = bacc.Bacc(None, target_bir_lowering=False, debug=False)

    x = nc.declare_dram_parameter("x", [NN, C], F32, isOutput=False)
    ea = nc.declare_dram_parameter("ea", [E, EC], F32, isOutput=False)
    dsts = nc.declare_dram_parameter("dsts", [128, SLOTS], I32, isOutput=False)
    gidx = nc.declare_dram_parameter("gidx", [128, SLOTS], I32, isOutput=False)
    pr = {}
    for nm, shp in (("dp_w1", [EC, 1]), ("dp_b1", [1, 1]), ("sc_w1", [C, 1]),
                    ("sc_b1", [1, 1]), ("dp_w2", [EC, 1]), ("dp_b2", [1, 1]),
                    ("sc_w2", [C, 1]), ("sc_b2", [1, 1]), ("out_w", [C, 1]),
                    ("out_b", [1, 1]), ("iota32", [1, 32])):
        pr[nm] = nc.declare_dram_parameter(nm, shp, F32, isOutput=False)
    outp = nc.declare_dram_parameter("out", [G, 1], F32, isOutput=True)
    dbg = {}
    if debug:
        for nm in ("d_proj", "d_w", "d_score1", "d_kept1", "d_m", "d_score2",
                   "d_kept2", "d_cnt", "d_cnt2", "d_compact1", "d_compact2"):
            shape = [128, SLOTS] if "compact" in nm or nm == "d_w" else [128, NCOL]
            if nm == "d_proj":
                shape = [128, NCOL * 3]
            if nm == "d_w":
                shape = [128, SLOTS * 2]
            dbg[nm] = nc.declare_dram_parameter(nm, shape, F32, isOutput=True)

    bounce = nc.dram_tensor("bounce", [NN], F32)

    with tile.TileContext(nc) as tc:
        with (
            tc.tile_pool(name="const", bufs=1) as cpool,
            tc.tile_pool(name="node", bufs=1) as npool,
            tc.tile_pool(name="edge", bufs=1) as epool,
            tc.tile_pool(name="work", bufs=2) as wpool,
            tc.tile_pool(name="ptr", bufs=3, space="PSUM") as pp_tr,
            tc.tile_pool(name="pmix", bufs=1, space="PSUM") as pmix,
        ):
            # ---------------- constants ----------------
            ident = cpool.tile([128, 128], F32)
            make_identity(nc, ident[:])
            ones_r = cpool.tile([1, 128], F32)
            nc.vector.memset(ones_r[:], 1.0)
            ones_c = cpool.tile([128, 1], F32)
            nc.vector.memset(ones_c[:], 1.0)

            iota_row = cpool.tile([1, 32], F32)
            nc.sync.dma_start(out=iota_row[:], in_=pr["iota32"][:])
            ps_small = pmix.tile([128, 32], F32, tag="small")
            nc.tensor.matmul(out=ps_small[:], lhsT=ones_r[:], rhs=iota_row[:],
                             start=True, stop=True)
            iota_t = cpool.tile([128, 32], F32)
            nc.scalar.copy(out=iota_t[:], in_=ps_small[:])

            def bcast_scalar(name, src):
                t0 = cpool.tile([1, 1], F32, tag=f"{name}_r")
                nc.sync.dma_start(out=t0[:], in_=src[:])
                psb = pmix.tile([128, 32], F32, tag="small")
                nc.tensor.matmul(out=psb[:, 0:1], lhsT=ones_r[:], rhs=t0[:],
                                 start=True, stop=True)
                t = cpool.tile([128, 1], F32, tag=f"{name}_b")
                nc.scalar.copy(out=t[:], in_=psb[:, 0:1])
                return t

            b1b = bcast_scalar("b1", pr["sc_b1"])
            b2b = bcast_scalar("b2", pr["sc_b2"])
            db1b = bcast_scalar("db1", pr["dp_b1"])
            db2b = bcast_scalar("db2", pr["dp_b2"])

            iota_b = cpool.tile([128, 32], BF16)
            nc.vector.tensor_copy(out=iota_b[:], in_=iota_t[:])
            iota_rep = cpool.tile([128, 64, 32], BF16)
            nc.vector.tensor_copy(
                out=iota_rep[:],
                in_=bass.AP(iota_b[:].tensor, iota_b[:].offset,
                            [list(iota_b[:].ap[0])] + [[0, 64], [1, 32]]))

            # static masks for per-graph bisection (8 partitions per graph),
            # derived from the identity to keep base partitions aligned
            gmask16 = cpool.tile([128, 16], F32)
            nc.vector.tensor_reduce(
                out=gmask16[:], in_=ident[:].rearrange("p (a b) -> p a b", b=8),
                axis=mybir.AxisListType.X, op=AL.add)
            gmask8 = gmask16[0:64, 0:G]          # [p, g] = 1 if p//8 == g
            ps_gT = pp_tr.tile([128, 128], F32, tag="ptr")
            nc.tensor.transpose(out=ps_gT[:G, 0:64], in_=gmask8,
                                identity=ident[:64, :64])
            gmask8T = cpool.tile([G, 64], F32)
            nc.vector.tensor_copy(out=gmask8T[:], in_=ps_gT[:G, 0:64])
            gsel = cpool.tile([64, G], F32)
            _ia = ident[0:64, :]
            nc.vector.tensor_copy(
                out=gsel[:],
                in_=bass.AP(_ia.tensor, _ia.offset, [list(_ia.ap[0])] + [[8, G]]))

            P3 = cpool.tile([128, 2, 3], F32)
            for cc in range(2):
                nc.sync.dma_start(out=P3[:, cc, 0:1], in_=pr["sc_w1"][128 * cc:128 * (cc + 1), :])
                nc.sync.dma_start(out=P3[:, cc, 1:2], in_=pr["sc_w2"][128 * cc:128 * (cc + 1), :])
                nc.sync.dma_start(out=P3[:, cc, 2:3], in_=pr["out_w"][128 * cc:128 * (cc + 1), :])

            wpats = []
            for b in range(3):
                wp = cpool.tile([128, 16], F32, tag=f"wpat{b}")
                nc.vector.memset(wp[:], 0.0)
                pstart = 0
                while pstart < 128:
                    jj, c0 = divmod(128 * b + pstart, EC)
                    run = min(128 - pstart, EC - c0)
                    for w, dpw in ((0, pr["dp_w1"]), (1, pr["dp_w2"])):
                        nc.sync.dma_start(
                            out=wp[pstart:pstart + run, 2 * jj + w:2 * jj + w + 1],
                            in_=dpw[c0:c0 + run, :])
                    pstart += run
                wpats.append(wp)
            wpats_b = []
            for b in range(3):
                wpb = cpool.tile([128, 16], BF16, tag=f"wpatb{b}")
                nc.vector.tensor_copy(out=wpb[:], in_=wpats[b][:])
                wpats_b.append(wpb)

            # ---------------- per-node tiles ----------------
            proj = npool.tile([128, NCOL, 3], F32)
            num_t = npool.tile([128, NCOL], F32)
            cnt_t = npool.tile([128, NCOL], F32)
            score1 = npool.tile([128, NCOL], F32)
            t1 = npool.tile([128, NCOL], F32)
            kept1 = npool.tile([128, NCOL], F32)
            xs1t = npool.tile([128, NCOL], F32)
            m_t = npool.tile([128, NCOL], F32)
            num2_t = npool.tile([128, NCOL], F32)
            cnt2_t = npool.tile([128, NCOL], F32)
            score2 = npool.tile([128, NCOL], F32)
            score2m = npool.tile([128, NCOL], F32)
            t2 = npool.tile([128, NCOL], F32)
            kept2 = npool.tile([128, NCOL], F32)
            negbig = npool.tile([128, NCOL], F32)
            nc.vector.memset(negbig[:], -1e30)


            for _rep in range(reps):
                # ---------------- x projection ----------------
                NT = NN // 128
                for bt in range(0, NT, 8):
                    psx = pmix.tile([128, 24], F32, tag="psx")
                    for ti in range(8):
                        tidx = bt + ti
                        xt = wpool.tile([128, C], F32, tag="xtile")
                        nc.sync.dma_start(out=xt[:], in_=x[128 * tidx:128 * (tidx + 1), :])
                        xT = wpool.tile([128, 2, 128], F32, tag="xT")
                        for cc in range(2):
                            pst = pp_tr.tile([128, 128], F32, tag="ptr")
                            nc.tensor.transpose(out=pst[:], in_=xt[:, 128 * cc:128 * (cc + 1)],
                                                identity=ident[:])
                            nc.scalar.copy(out=xT[:, cc, :], in_=pst[:])
                        for cc in range(2):
                            nc.tensor.matmul(out=psx[:, 3 * ti:3 * (ti + 1)],
                                             lhsT=xT[:, cc, :], rhs=P3[:, cc, :],
                                             start=(cc == 0), stop=(cc == 1))
                    nc.vector.tensor_copy(
                        out=proj[:, bt:bt + 8, :].rearrange("p a b -> p (a b)"),
                        in_=psx[:])

                nc.vector.tensor_scalar(out=xs1t[:], in0=proj[:, :, 0], scalar1=b1b[:, 0:1],
                                        scalar2=None, op0=AL.add)

                # ---------------- ea projection ----------------
                Wboth = epool.tile([128, SLOTS, 2], F32)
                eaf = ea.rearrange("e c -> (e c)")
                for t4 in range(0, E // 1024, 4):
                    psw = pmix.tile([128, 64], F32, tag="psw")
                    for ti in range(4):
                        tg = t4 + ti
                        reg = wpool.tile([128, 384], F32, tag="eareg")
                        src = bass.AP(eaf.tensor, eaf.offset + 1024 * tg * EC,
                                      [[8 * EC, 128], [1, 384]])
                        nc.sync.dma_start(out=reg[:], in_=src)
                        for b in range(3):
                            pst = pp_tr.tile([128, 128], F32, tag="ptr")
                            nc.tensor.transpose(out=pst[:], in_=reg[:, 128 * b:128 * (b + 1)],
                                                identity=ident[:])
                            tsb = wpool.tile([128, 128], BF16, tag="tsb")
                            nc.vector.tensor_copy(out=tsb[:], in_=pst[:])
                            nc.tensor.matmul(out=psw[:, 16 * ti:16 * (ti + 1)],
                                             lhsT=tsb[:], rhs=wpats_b[b][:],
                                             start=(b == 0), stop=(b == 2))
                    nc.scalar.copy(
                        out=Wboth[:, 8 * t4:8 * (t4 + 4), :].rearrange("p a b -> p (a b)"),
                        in_=psw[:])

                W1b = epool.tile([128, SLOTS], BF16)
                W2b = epool.tile([128, SLOTS], BF16)
                nc.vector.tensor_scalar(out=W1b[:], in0=Wboth[:, :, 0], scalar1=db1b[:, 0:1],
                                        scalar2=None, op0=AL.add)
                nc.vector.tensor_scalar(out=W2b[:], in0=Wboth[:, :, 1], scalar1=db2b[:, 0:1],
                                        scalar2=None, op0=AL.add)
                if debug:
                    nc.sync.dma_start(out=dbg["d_w"][:, 0:SLOTS], in_=W1b[:])
                    nc.sync.dma_start(out=dbg["d_w"][:, SLOTS:2 * SLOTS], in_=W2b[:])

                # ---------------- dst hi/lo ----------------
                dst32 = epool.tile([128, SLOTS], I32, tag="i32a")
                nc.sync.dma_start(out=dst32[:], in_=dsts[:])
                for g in range(G):
                    sl = slice(128 * g, 128 * (g + 1))
                    nc.vector.tensor_scalar(out=dst32[:, sl], in0=dst32[:, sl],
                                            scalar1=NPG * g, scalar2=None, op0=AL.subtract)
                hi_f = epool.tile([128, SLOTS], BF16)
                lo_f = epool.tile([128, SLOTS], BF16)
                tmp_i = epool.tile([128, SLOTS], I32, tag="i32b")
                nc.vector.tensor_scalar(out=tmp_i[:], in0=dst32[:], scalar1=5, scalar2=None,
                                        op0=AL.logical_shift_right)
                nc.vector.tensor_copy(out=hi_f[:], in_=tmp_i[:])
                nc.vector.tensor_scalar(out=tmp_i[:], in0=dst32[:], scalar1=31, scalar2=None,
                                        op0=AL.bitwise_and)
                nc.vector.tensor_copy(out=lo_f[:], in_=tmp_i[:])

                gidx32 = epool.tile([128, SLOTS], I32, tag="i32a")
                nc.sync.dma_start(out=gidx32[:], in_=gidx[:])
                gidx16 = epool.tile([128, SLOTS], I16)
                nc.vector.tensor_scalar(out=gidx16[:], in0=gidx32[:], scalar1=0, scalar2=None,
                                        op0=AL.add)

                table = epool.tile([128, NN], F32)
                nc.vector.memset(table[:], 0.0)
                gout = epool.tile([128, 16384], F32)
                compact = epool.tile([128, SLOTS], F32)

                def build_table(src_tile):
                    pst = pp_tr.tile([128, 128], F32, tag="ptr")
                    nc.tensor.transpose(out=pst[:NCOL, :], in_=src_tile[:], identity=ident[:])
                    mT = wpool.tile([NCOL, 128], F32, tag="mT")
                    nc.vector.tensor_copy(out=mT[:], in_=pst[:NCOL, :])
                    nc.sync.dma_start(out=bounce.rearrange("(a b) -> a b", a=NCOL), in_=mT[:])
                    for k in range(8):
                        nc.sync.dma_start(out=table[16 * k:16 * k + 1, :],
                                          in_=bounce[None, :])

                def gather_compact():
                    nc.gpsimd.ap_gather(gout[:], table[:], gidx16[:],
                                        channels=128, num_elems=NN, d=1,
                                        num_idxs=16384)
                    for b in range(128):
                        pst = pp_tr.tile([128, 128], F32, tag="ptr")
                        nc.tensor.transpose(out=pst[:],
                                            in_=gout[:, 128 * b:128 * (b + 1)],
                                            identity=ident[:])
                        csrc = _ap(pst, 0, [[16, 8]])
                        cdst = _ap(compact, b, [[128, 8]])
                        nc.vector.tensor_copy(out=cdst, in_=csrc)

                def bilinear(msg_tile, cnt_src_tile, num_out, cnt_out):
                    for g in range(G):
                        psb = pmix.tile([64, 32], F32, tag="psb")
                        for hh in range(2):
                            s0 = 128 * g + 64 * hh
                            TH = wpool.tile([128, 64, 64], BF16, tag="TH")
                            L = wpool.tile([128, 64, 32], BF16, tag="L")
                            lo_ap = _ap(lo_f, s0, [[1, 64], [0, 32]])
                            hi_ap = _ap(hi_f, s0, [[1, 64], [0, 32]])
                            # materialize broadcast operands on ACT so the DVE
                            # ops see step-1 bf16 and run in 2x mode
                            lo_rep = wpool.tile([128, 64, 32], BF16, tag="lorep")
                            nc.scalar.copy(out=lo_rep[:], in_=lo_ap)
                            hi_rep = wpool.tile([128, 64, 32], BF16, tag="hirep")
                            nc.scalar.copy(out=hi_rep[:], in_=hi_ap)
                            msg_ap = _ap(msg_tile, s0, [[1, 64], [0, 32]])
                            nc.vector.tensor_tensor(out=L[:], in0=lo_rep[:],
                                                    in1=iota_rep[:], op=AL.is_equal)
                            nc.vector.tensor_tensor(out=TH[:, :, 32:64],
                                                    in0=hi_rep[:], in1=iota_rep[:],
                                                    op=AL.is_equal)
                            nc.vector.tensor_tensor(out=TH[:, :, 0:32],
                                                    in0=TH[:, :, 32:64], in1=msg_ap,
                                                    op=AL.mult)
                            if cnt_src_tile is not None:
                                cs_ap = _ap(cnt_src_tile, s0, [[1, 64], [0, 32]])
                                nc.vector.tensor_tensor(out=TH[:, :, 32:64],
                                                        in0=TH[:, :, 32:64], in1=cs_ap,
                                                        op=AL.mult)
                            for si in range(64):
                                nc.tensor.matmul(out=psb[:], lhsT=TH[:, si, :],
                                                 rhs=L[:, si, :],
                                                 start=(hh == 0 and si == 0),
                                                 stop=(hh == 1 and si == 63))
                        sb1 = wpool.tile([64, 32], F32, tag="sb1")
                        nc.vector.tensor_copy(out=sb1[:], in_=psb[:])
                        pst2 = pmix.tile([32, 64], F32, tag="ptr2")
                        nc.tensor.transpose(out=pst2[:], in_=sb1[:], identity=ident[:64, :64])
                        sb2 = wpool.tile([32, 64], F32, tag="sb2")
                        nc.vector.tensor_copy(out=sb2[:], in_=pst2[:])
                        # sb2[lo, hi] : cols 0:32 -> num, 32:64 -> cnt
                        for (col0, dstt) in ((0, num_out), (32, cnt_out)):
                            for h4 in range(4):
                                din = _ap(sb2, col0 + h4, [[4, 8]])
                                dout = dstt[32 * h4:32 * (h4 + 1), 8 * g:8 * g + 8]
                                nc.sync.dma_start(out=dout, in_=din)

                def mean_guard(numt, cntt, out):
                    cm = wpool.tile([128, NCOL], F32, tag="cm")
                    nc.vector.tensor_scalar_max(cm[:], cntt[:], 1.0)
                    dv = wpool.tile([128, NCOL], F32, tag="dv")
                    nc.vector.reciprocal(out=cm[:], in_=cm[:])
                    nc.vector.tensor_tensor(out=dv[:], in0=numt[:], in1=cm[:], op=AL.mult)
                    mk = wpool.tile([128, NCOL], I8, tag="mk")
                    nc.vector.tensor_scalar(out=mk[:], in0=cntt[:], scalar1=0.0, scalar2=None,
                                            op0=AL.is_gt)
                    zz = wpool.tile([128, NCOL], F32, tag="zz")
                    nc.vector.memset(zz[:], 0.0)
                    nc.vector.select(out=out[:], mask=mk[:], on_true=dv[:], on_false=zz[:])

                def thresholds_bisect(sc_tile, target, tg, iters=36):
                    """Exact per-graph count-target threshold via bisection.

                    sc_tile [128, NCOL] f32 node-major scores -> tau tile
                    [128, G] (per-graph threshold replicated down partitions).
                    Invariant: count(>= lo) >= target > count(>= hi)."""
                    pst = pp_tr.tile([128, 128], F32, tag="ptr")
                    nc.tensor.transpose(out=pst[:NCOL, :], in_=sc_tile[:],
                                        identity=ident[:])
                    scT = wpool.tile([64, 128], F32, tag=f"scT{tg}")
                    nc.vector.tensor_copy(out=scT[:], in_=pst[:NCOL, :])
                    lo = wpool.tile([64, 1], F32, tag=f"lo{tg}")
                    hi = wpool.tile([64, 1], F32, tag=f"hi{tg}")
                    mid = wpool.tile([64, 1], F32, tag=f"mid{tg}")
                    ge = wpool.tile([64, 128], F32, tag=f"ge{tg}")
                    pc = wpool.tile([64, 1], F32, tag=f"pc{tg}")
                    fl8 = wpool.tile([8, 1], F32, tag=f"fl8{tg}")
                    d1 = wpool.tile([64, 1], F32, tag=f"d1{tg}")
                    nc.vector.memset(lo[:], -64.0)
                    nc.vector.memset(hi[:], 64.0)
                    for _it in range(iters):
                        nc.vector.tensor_tensor(out=mid[:], in0=lo[:], in1=hi[:],
                                                op=AL.add)
                        nc.vector.tensor_scalar(out=mid[:], in0=mid[:], scalar1=0.5,
                                                scalar2=None, op0=AL.mult)
                        nc.vector.tensor_scalar(out=ge[:], in0=scT[:],
                                                scalar1=mid[:, 0:1], scalar2=None,
                                                op0=AL.is_ge)
                        nc.vector.tensor_reduce(out=pc[:], in_=ge[:],
                                                axis=mybir.AxisListType.X, op=AL.add)
                        psc = pmix.tile([128, 32], F32, tag="small")
                        nc.tensor.matmul(out=psc[:G, 0:1], lhsT=gmask8[:], rhs=pc[:],
                                         start=True, stop=True)
                        nc.vector.tensor_scalar(out=fl8[:], in0=psc[:G, 0:1],
                                                scalar1=target - 0.5, scalar2=None,
                                                op0=AL.is_ge)
                        psf = pp_tr.tile([128, 128], F32, tag="ptr")
                        nc.tensor.matmul(out=psf[:64, 0:1], lhsT=gmask8T[:],
                                         rhs=fl8[:], start=True, stop=True)
                        # fl=1 -> lo=mid ; fl=0 -> hi=mid
                        nc.vector.tensor_tensor(out=d1[:], in0=mid[:], in1=lo[:],
                                                op=AL.subtract)
                        nc.vector.tensor_tensor(out=d1[:], in0=d1[:],
                                                in1=psf[:64, 0:1], op=AL.mult)
                        nc.vector.tensor_tensor(out=lo[:], in0=lo[:], in1=d1[:],
                                                op=AL.add)
                        nc.vector.tensor_tensor(out=d1[:], in0=hi[:], in1=mid[:],
                                                op=AL.subtract)
                        nc.vector.tensor_tensor(out=d1[:], in0=d1[:],
                                                in1=psf[:64, 0:1], op=AL.mult)
                        nc.vector.tensor_tensor(out=hi[:], in0=mid[:], in1=d1[:],
                                                op=AL.add)
                    # tau row [1, G] then broadcast to [128, G]
                    psr = pmix.tile([128, 32], F32, tag="small")
                    nc.tensor.matmul(out=psr[:1, 0:G], lhsT=lo[:], rhs=gsel[:],
                                     start=True, stop=True)
                    trow = wpool.tile([1, G], F32, tag=f"trow{tg}")
                    nc.vector.tensor_copy(out=trow[:], in_=psr[:1, 0:G])
                    psb = pmix.tile([128, 32], F32, tag="small")
                    nc.tensor.matmul(out=psb[:, 0:G], lhsT=ones_r[:], rhs=trow[:],
                                     start=True, stop=True)
                    tt = wpool.tile([128, G], F32, tag=tg)
                    nc.scalar.copy(out=tt[:], in_=psb[:, 0:G])
                    return tt

                def ge_mask(sc_tile, tau_tile, out):
                    for g in range(G):
                        nc.vector.tensor_scalar(out=out[:, 8 * g:8 * (g + 1)],
                                                in0=sc_tile[:, 8 * g:8 * (g + 1)],
                                                scalar1=tau_tile[:, g:g + 1], scalar2=None,
                                                op0=AL.is_ge)

                # ================= LAYER 1 =================
                if stage < 2:
                    continue
                build_table(xs1t)
                gather_compact()
                if debug:
                    nc.sync.dma_start(out=dbg["d_compact1"][:], in_=compact[:])
                if stage < 3:
                    continue
                compact_b = epool.tile([128, SLOTS], BF16, tag="cb")
                nc.vector.tensor_copy(out=compact_b[:], in_=compact[:])
                msg1 = epool.tile([128, SLOTS], BF16, tag="msg")
                nc.vector.tensor_tensor(out=msg1[:], in0=compact_b[:], in1=W1b[:], op=AL.mult)
                bilinear(msg1, None, num_t, cnt_t)
                if stage < 4:
                    continue
                mean_guard(num_t, cnt_t, score1)
                tau1 = thresholds_bisect(score1, float(K1), "tau1")
                ge_mask(score1, tau1, kept1)
                nc.scalar.activation(out=t1[:], in_=score1[:], func=ACTF.Tanh)
                nc.vector.tensor_tensor(out=m_t[:], in0=proj[:, :, 1], in1=t1[:], op=AL.mult)
                nc.vector.tensor_scalar(out=m_t[:], in0=m_t[:], scalar1=b2b[:, 0:1],
                                        scalar2=None, op0=AL.add)
                nc.vector.tensor_tensor(out=m_t[:], in0=m_t[:], in1=kept1[:], op=AL.mult)

                # ================= LAYER 2 =================
                if stage < 5:
                    continue
                build_table(m_t)
                gather_compact()
                if debug:
                    nc.sync.dma_start(out=dbg["d_compact2"][:], in_=compact[:])
                compact_b2 = epool.tile([128, SLOTS], BF16, tag="cb")
                nc.vector.tensor_copy(out=compact_b2[:], in_=compact[:])
                msg2 = epool.tile([128, SLOTS], BF16, tag="msg")
                nc.vector.tensor_tensor(out=msg2[:], in0=compact_b2[:], in1=W2b[:], op=AL.mult)
                ksrc = epool.tile([128, SLOTS], BF16)
                nc.vector.tensor_scalar(out=ksrc[:], in0=compact_b2[:], scalar1=0.0,
                                        scalar2=None, op0=AL.not_equal)
                if stage < 6:
                    continue
                bilinear(msg2, ksrc, num2_t, cnt2_t)
                mean_guard(num2_t, cnt2_t, score2)
                kept1_i8 = wpool.tile([128, NCOL], I8, tag="k1i8")
                nc.vector.tensor_copy(out=kept1_i8[:], in_=kept1[:])
                nc.vector.select(out=score2m[:], mask=kept1_i8[:], on_true=score2[:],
                                 on_false=negbig[:])
                tau2 = thresholds_bisect(score2m, float(K2), "tau2")
                ge_mask(score2m, tau2, kept2)
                nc.vector.tensor_tensor(out=kept2[:], in0=kept2[:], in1=kept1[:], op=AL.mult)
                nc.scalar.activation(out=t2[:], in_=score2[:], func=ACTF.Tanh)

                # ================= FINAL =================
                acc = wpool.tile([128, NCOL], F32, tag="acc")
                nc.vector.tensor_tensor(out=acc[:], in0=kept2[:], in1=t2[:], op=AL.mult)
                nc.vector.tensor_scalar(out=acc[:], in0=acc[:], scalar1=1.0, scalar2=None,
                                        op0=AL.add)
                nc.vector.tensor_tensor(out=acc[:], in0=acc[:], in1=t1[:], op=AL.mult)
                nc.vector.tensor_tensor(out=acc[:], in0=acc[:], in1=kept1[:], op=AL.mult)
                nc.vector.tensor_scalar(out=acc[:], in0=acc[:], scalar1=1.0, scalar2=None,
                                        op0=AL.add)
                nc.vector.tensor_tensor(out=acc[:], in0=acc[:], in1=proj[:, :, 2],
                                        op=AL.mult)
                part = wpool.tile([128, G], F32, tag="part")
                nc.vector.tensor_reduce(out=part[:],
                                        in_=acc[:].rearrange("p (g c) -> p g c", g=G),
                                        axis=mybir.AxisListType.X, op=AL.add)
                psS = pmix.tile([128, 32], F32, tag="small")
                nc.tensor.matmul(out=psS[:1, 0:G], lhsT=ones_c[:], rhs=part[:],
                                 start=True, stop=True)
                outb_r = cpool.tile([1, 1], F32, tag="outb")
                nc.sync.dma_start(out=outb_r[:], in_=pr["out_b"][:])
                sres = wpool.tile([1, G], F32, tag="sres")
                nc.scalar.activation(out=sres[:], in_=psS[:1, 0:G], func=ACTF.Sigmoid,
                                     bias=outb_r[:, 0:1])
                nc.sync.dma_start(out=outp[:, 0:1].rearrange("a b -> b a"), in_=sres[:])

            if debug:
                nc.sync.dma_start(out=dbg["d_proj"][:],
                                  in_=proj[:].rearrange("p a b -> p (a b)"))
                for nm, tt in (("d_score1", score1), ("d_kept1", kept1), ("d_m", m_t),
                               ("d_score2", score2), ("d_kept2", kept2),
                               ("d_cnt", cnt_t), ("d_cnt2", cnt2_t)):
                    nc.sync.dma_start(out=dbg[nm][:], in_=tt[:])

    nc.finalize()
    return nc


# ---------------------------------------------------------------------------
_E_OF_SLOT = None
_GIDX_EDGE = None


def _slot_maps():
    global _E_OF_SLOT, _GIDX_EDGE
    if _E_OF_SLOT is None:
        p = np.arange(128)[:, None]
        s = np.arange(SLOTS)[None, :]
        _E_OF_SLOT = 1024 * (s // 8) + 8 * p + (s % 8)
        j = np.arange(EPG)
        b = j // 128
        pp_ = j % 128
        _GIDX_EDGE = 1024 * (b // 8) + 8 * pp_ + (b % 8)
    return _E_OF_SLOT, _GIDX_EDGE


def make_core_inputs(inputs, core):
    e_of_slot, gidx_edge = _slot_maps()
    n0 = core * NN
    e0 = core * E
    src = np.asarray(inputs["edge_index"][0, e0:e0 + E], np.int64) - n0
    dst = np.asarray(inputs["edge_index"][1, e0:e0 + E], np.int64) - n0
    gi = np.empty((128, SLOTS), np.int32)
    jj = np.arange(EPG)
    for k in range(8):
        gi[16 * k + jj % 16, jj // 16] = src[EPG * k + gidx_edge]
    d = dict(
        x=np.ascontiguousarray(inputs["x"][n0:n0 + NN], dtype=np.float32),
        ea=np.ascontiguousarray(inputs["edge_attr"][e0:e0 + E], dtype=np.float32),
        dsts=dst[e_of_slot].astype(np.int32),
        gidx=gi,
        iota32=np.arange(32, dtype=np.float32).reshape(1, 32),
    )
    for nm, shp in (("dp_w1", (EC, 1)), ("dp_b1", (1, 1)), ("sc_w1", (C, 1)),
                    ("sc_b1", (1, 1)), ("dp_w2", (EC, 1)), ("dp_b2", (1, 1)),
                    ("sc_w2", (C, 1)), ("sc_b2", (1, 1)), ("out_w", (C, 1)),
                    ("out_b", (1, 1))):
        d[nm] = np.asarray(inputs[nm], np.float32).reshape(shp)
    return d


_NC_CACHE = None


def kernel(**inputs):
    global _NC_CACHE
    if _NC_CACHE is None:
        _NC_CACHE = build_program()
    in_maps = [make_core_inputs(inputs, c) for c in range(8)]
    res = run_bass_kernel_spmd(_NC_CACHE, in_maps, list(range(8)))
    return np.concatenate([res.results[c]["out"] for c in range(8)], axis=0)

